# revision 1
# baseline (speedup 1.0000x reference)
"""Trainium2 Bass kernel for nn_BlurLayer (B=128, 224x224x3, per-sample
rotated-line motion blur, SAME depthwise conv).

Self-contained: kernel(**inputs) -> np.ndarray. Shards the batch over 8
NeuronCores (pure data parallel: 16 samples per core), compiles + runs one
SPMD Bass program via concourse.bass_utils.run_bass_kernel_spmd, gathers
the full output.

Method: the rotated blur kernel's nonzero taps all equal 1/size and form a
digitized line. Taps grouped by kernel column (or row, on the transposed
image, whichever span is smaller) give contiguous runs -> banded 0/1
weight matrices contracted over image rows on the PE (PSUM-accumulated
float32r matmuls); the horizontal component is one per-sample alignment
done with a dynamic-offset (register-driven) DVE copy, after which all
matmul access patterns are static. Scaling by 1/size is fused into the
PSUM->SBUF copy on the Scalar engine. Samples are span-load-balanced
across cores and heavy/light slots interleaved for pipeline overlap.
"""


import concourse.mybir as mybir

def split_sync_waits(nc, max_waits=1):
    n_split = 0
    for fn in nc.m.functions:
        for blk in fn.blocks:
            new_insts = []
            for inst in blk.instructions:
                si = inst.sync_info
                waits = list(si.on_wait) if (si and si.on_wait) else []
                if len(waits) > max_waits:
                    keep = waits[-max_waits:]
                    extra = waits[:-max_waits]
                    for j, w in enumerate(extra):
                        n_split += 1
                        nop = mybir.InstNoOp(
                            name=f"{inst.name}-waitsplit-{j}",
                            engine=inst.engine,
                            ins=[], outs=[],
                            sync_info=mybir.SyncInfo(on_wait=[w], on_update=[]),
                        )
                        new_insts.append(nop)
                    inst.sync_info = mybir.SyncInfo(on_wait=keep, on_update=list(si.on_update or []))
                new_insts.append(inst)
            blk.instructions = new_insts
    return n_split



import math

import numpy as np

MAXK = 32
H = W = 224
C = 3
HW_PAD = 48                 # zero margin (elems) each side: 16 px * 3 ch
WPAD = W * C + 2 * HW_PAD   # 768
NHALF = (W // 2) * C        # 336
PAD_LO = (MAXK - 1) // 2    # 15


# ---------------------------------------------------------------- host math
def rotate_nearest_np(img, rad):
    K = img.shape[0]
    cos, sin = np.cos(rad), np.sin(rad)
    coords = np.arange(K, dtype=np.float32)
    yy, xx = np.meshgrid(coords, coords, indexing="ij")
    e = np.float32(K - 1)
    x_off = (e - (cos * e - sin * e)) * 0.5
    y_off = (e - (sin * e + cos * e)) * 0.5
    sx = cos * xx - sin * yy + x_off
    sy = sin * xx + cos * yy + y_off
    ix = np.round(sx).astype(np.int32)
    iy = np.round(sy).astype(np.int32)
    valid = (ix >= 0) & (ix < K) & (iy >= 0) & (iy < K)
    g = img[np.clip(iy, 0, K - 1), np.clip(ix, 0, K - 1)]
    return np.where(valid, g, np.float32(0.0))


def _col_groups(ker):
    """Group nonzero taps of `ker` by column -> [(kx, klo, khi)], splitting
    any non-contiguous run."""
    ys, xs = np.nonzero(ker)
    groups = []
    for kx in np.unique(xs):
        run = np.sort(ys[xs == kx])
        start = prev = int(run[0])
        for v in run[1:]:
            v = int(v)
            if v == prev + 1:
                prev = v
            else:
                groups.append((int(kx), start, prev))
                start = prev = v
        groups.append((int(kx), start, prev))
    return groups


def _span(groups):
    if not groups:
        return 1
    kxs = [t[0] for t in groups]
    return max(kxs) - min(kxs) + 1


def sample_plan(tbl_ch0, amt_b, ang_b):
    """-> (scale, groups, transposed). groups are column-groups of the
    (possibly transposed) kernel; transposed chooses the smaller span."""
    rad = np.float32(ang_b * math.pi / 180.0)
    ker = rotate_nearest_np(tbl_ch0[amt_b], rad)
    ys, xs = np.nonzero(ker)
    if len(ys) == 0:
        return np.float32(0.0), [], False
    scale = float(ker[ys[0], xs[0]])
    g_n = _col_groups(ker)
    g_t = _col_groups(ker.T)
    if _span(g_t) < _span(g_n):
        return np.float32(scale), g_t, True
    return np.float32(scale), g_n, False


def band_matrices(klo, khi):
    """W0 [128,112]: out rows 0..111 vs img rows 0..127 (tile row r = img
    row r, band r-y in [klo-15, khi-15]); W1: out rows 112..223 vs img rows
    96..223 (tile row r = img row 96+r, band r-y in [klo+1, khi+1])."""
    r = np.arange(128)[:, None]
    y = np.arange(112)[None, :]
    d = r - y
    w0 = ((d >= klo - PAD_LO) & (d <= khi - PAD_LO)).astype(np.uint8)
    w1 = ((d >= klo + 1) & (d <= khi + 1)).astype(np.uint8)
    return w0, w1


def prepare_host(x, kernels_table, amt, angles, n_cores=8):
    B = x.shape[0]
    assert B % n_cores == 0
    slots = B // n_cores
    tbl_ch0 = np.ascontiguousarray(kernels_table[:, :, :, 0])

    scales = np.zeros(B, np.float32)
    groups = []
    transposed = np.zeros(B, bool)
    spans = np.zeros(B, np.int64)
    for b in range(B):
        s, g, tr = sample_plan(tbl_ch0, int(amt[b]), int(angles[b]))
        scales[b] = s
        groups.append(g)
        transposed[b] = tr
        spans[b] = _span(g)

    # load balance on span: sort desc, 8 consecutive samples -> one slot.
    # Then interleave heavy/light chunks (light first) so small slots'
    # fixed latency hides behind big slots' PE work instead of clustering
    # at the kernel tail.
    order = np.argsort(-spans, kind="stable")
    asg = order.reshape(slots, n_cores)
    ileave = []
    lo, hi = 0, slots - 1
    while lo <= hi:
        if lo != hi:
            ileave.extend([hi, lo])
        else:
            ileave.append(lo)
        lo += 1
        hi -= 1
    asg = asg[np.array(ileave)]
    gmax = np.array([max(1, spans[asg[j]].max()) for j in range(slots)])
    col_base = np.concatenate([[0], np.cumsum(gmax * 224)])[:-1]
    totcols = int((gmax * 224).sum())

    in_maps = []
    mapping = np.zeros((n_cores, slots), np.int64)
    for c in range(n_cores):
        ximg = np.zeros((slots, H, WPAD), np.float32)
        wts = np.zeros((128, totcols), np.uint8)
        offs = np.full((1, slots), 3, np.int32)
        scl = np.zeros((128, slots), np.float32)
        for j in range(slots):
            b = int(asg[j, c])
            G = int(gmax[j])
            mapping[c, j] = b
            img = x[b].transpose(1, 0, 2) if transposed[b] else x[b]
            ximg[j, :, HW_PAD:HW_PAD + W * C] = np.ascontiguousarray(img).reshape(H, W * C)
            scl[:, j] = scales[b]
            if groups[b]:
                kxmin = min(t[0] for t in groups[b])
                base_kx = min(kxmin, MAXK - G)
            else:
                base_kx = 0
            offs[0, j] = 3 * base_kx + 3
            for kx, klo, khi in groups[b]:
                i = kx - base_kx
                assert 0 <= i < G, (b, kx, base_kx, G)
                w0, w1 = band_matrices(klo, khi)
                base = col_base[j] + 224 * i
                wts[:, base:base + 112] += w0
                wts[:, base + 112:base + 224] += w1
        in_maps.append({"ximg": ximg, "wts": wts, "offs": offs, "scl": scl})

    meta = {
        "slots": slots,
        "gmax": [int(v) for v in gmax],
        "col_base": [int(v) for v in col_base],
        "totcols": totcols,
        "mapping": mapping,
        "transposed": transposed,
    }
    return meta, in_maps


# ---------------------------------------------------------------- device IR
def build_program(meta, dtype_name="float32r"):
    import concourse.bacc as bacc
    import concourse.bass as bass
    import concourse.mybir as mybir
    from concourse.bass_types import RegisterHandles
    from concourse.tile import TileContext

    ET = mybir.EngineType
    fdt = getattr(mybir.dt, dtype_name)
    slots = meta["slots"]
    gmax = meta["gmax"]
    col_base = meta["col_base"]

    nc = bacc.Bacc("TRN2")
    ximg = nc.dram_tensor("ximg", [slots, H, WPAD], fdt, kind="ExternalInput")
    wts = nc.dram_tensor("wts", [128, meta["totcols"]], mybir.dt.uint8,
                         kind="ExternalInput")
    offs = nc.dram_tensor("offs", [1, slots], mybir.dt.int32,
                          kind="ExternalInput")
    scl = nc.dram_tensor("scl", [128, slots], mybir.dt.float32,
                         kind="ExternalInput")
    out = nc.dram_tensor("out", [slots, H, W * C], mybir.dt.float32,
                         kind="ExternalOutput")

    with TileContext(nc) as tc:
        with tc.tile_pool(name="const", bufs=1) as cpool, \
             tc.tile_pool(name="img", bufs=5) as ipool, \
             tc.tile_pool(name="shf", bufs=5) as spool, \
             tc.tile_pool(name="wt", bufs=3) as wpool, \
             tc.tile_pool(name="res", bufs=5) as rpool, \
             tc.tile_pool(name="ps0", bufs=2, space="PSUM") as pp00, \
             tc.tile_pool(name="ps1", bufs=2, space="PSUM") as pp01, \
             tc.tile_pool(name="ps2", bufs=2, space="PSUM") as pp10, \
             tc.tile_pool(name="ps3", bufs=2, space="PSUM") as pp11:
            ot = cpool.tile([1, slots], mybir.dt.int32)
            st = cpool.tile([128, slots], mybir.dt.float32)
            nc.sync.dma_start(out=ot, in_=offs[:, :])
            nc.sync.dma_start(out=st, in_=scl[:, :])

            rD = nc.alloc_register(ET.DVE, "offD")

            for j in range(slots):
                G = gmax[j]
                SHW = 3 * (G - 1) + W * C  # shifted tile width
                t0 = ipool.tile([128, WPAD], fdt, tag="t0", name="t0")
                t1 = ipool.tile([128, WPAD], fdt, tag="t1", name="t1")
                nc.sync.dma_start(out=t0, in_=ximg[j, 0:128, :])
                nc.sync.dma_start(out=t1, in_=ximg[j, 96:224, :])
                wt = wpool.tile([128, 224 * G], fdt, tag="wt", name="wt")
                nc.gpsimd.dma_start(
                    out=wt, in_=wts[:, col_base[j]:col_base[j] + 224 * G])

                nc.engines[ET.DVE].reg_load(rD, ot[0:1, j:j + 1])
                v = nc.snap(RegisterHandles([rD]), donate=True,
                            min_val=3, max_val=3 * (MAXK - G) + 3)

                sh0 = spool.tile([128, SHW], fdt, tag="sh0", name="sh0")
                sh1 = spool.tile([128, SHW], fdt, tag="sh1", name="sh1")
                nc.vector.tensor_copy(out=sh0, in_=t0[:, bass.ds(v, SHW)])
                nc.vector.tensor_copy(out=sh1, in_=t1[:, bass.ds(v, SHW)])

                psums = [[pp00.tile([112, NHALF], mybir.dt.float32,
                                    tag="ps00", name="ps00"),
                          pp01.tile([112, NHALF], mybir.dt.float32,
                                    tag="ps01", name="ps01")],
                         [pp10.tile([112, NHALF], mybir.dt.float32,
                                    tag="ps10", name="ps10"),
                          pp11.tile([112, NHALF], mybir.dt.float32,
                                    tag="ps11", name="ps11")]]
                tiles = [sh0, sh1]
                for g in range(G):
                    first = g == 0
                    last = g == G - 1
                    for hb in (0, 1):
                        lhsT = wt[:, 224 * g + 112 * hb:224 * g + 112 * hb + 112]
                        for wh in (0, 1):
                            s0 = 3 * g + NHALF * wh
                            nc.tensor.matmul(psums[hb][wh], lhsT=lhsT,
                                             rhs=tiles[hb][:, s0:s0 + NHALF],
                                             start=first, stop=last)
                for hb in (0, 1):
                    rt = rpool.tile([112, W * C], mybir.dt.float32,
                                    tag=f"r{hb}", name=f"r{hb}")
                    for wh in (0, 1):
                        nc.scalar.activation(
                            out=rt[:, NHALF * wh:NHALF * (wh + 1)],
                            in_=psums[hb][wh],
                            func=mybir.ActivationFunctionType.Copy,
                            scale=st[0:112, j:j + 1])
                    nc.sync.dma_start(out=out[j, 112 * hb:112 * (hb + 1), :],
                                      in_=rt)
    return nc


def run_cores(meta, in_maps, dtype_name="float32r", trace=False):
    from concourse.bass_utils import run_bass_kernel_spmd

    nc = build_program(meta, dtype_name)
    nc.compile()
    split_sync_waits(nc)
    res = run_bass_kernel_spmd(nc, in_maps, core_ids=list(range(len(in_maps))),
                               trace=trace)
    return res


def unshard(meta, results):
    B = meta["mapping"].size
    out = np.zeros((B, H, W, C), np.float32)
    for c, r in enumerate(results):
        o = r["out"].reshape(meta["slots"], H, W, C)
        for j in range(meta["slots"]):
            b = meta["mapping"][c, j]
            img = o[j]
            if meta["transposed"][b]:
                img = img.transpose(1, 0, 2)
            out[b] = img
    return out


def kernel(x, kernels_table, amt, angles):
    x = np.asarray(x, np.float32)
    kernels_table = np.asarray(kernels_table, np.float32)
    amt = np.asarray(amt)
    angles = np.asarray(angles)
    meta, in_maps = prepare_host(x, kernels_table, amt, angles)
    res = run_cores(meta, in_maps)
    return unshard(meta, res.results)



# revision 2
# speedup vs baseline: 1.4773x; 1.4773x over previous
"""Trainium2 Bass kernel for nn_BlurLayer (B=128, 224x224x3, per-sample
rotated-line motion blur, SAME depthwise conv).

Self-contained: kernel(**inputs) -> np.ndarray. Shards the batch over 8
NeuronCores (pure data parallel: 16 samples per core), compiles + runs one
SPMD Bass program via concourse.bass_utils.run_bass_kernel_spmd, gathers
the full output.

Method: the rotated blur kernel's taps (all equal 1/size) are grouped by
kernel column; each column group becomes a banded 0/1 weight matrix
contracted over image rows on the PE (PSUM-accumulated bf16 matmuls), and
the horizontal offset between groups is a static +3-elem slide of the
moving operand. All per-sample alignment (base column, transpose, flip,
integer shear) is baked into host-side data placement, so the device
program is fully static with no dynamic registers and no on-device shift
copies. Diagonal-ish lines are sheared by one column per row on the host
(making them near-vertical, shrinking the group count); sheared outputs
are written in sheared coordinates (wider rows) and unsheared on the
host. The 1/size scale is folded into the image pixels on the host.
Images/weights/outputs move as bf16/fp8/bf16 to cut HBM traffic; PSUM
accumulates in fp32. Slots are packed by a local search minimizing
sum-over-slots of (psum width x max group count).
"""

import math

import numpy as np

import concourse.mybir as mybir

MAXK = 32
H = W = 224
C = 3

WQ_UN = 224            # psum width in pixels per row-half, unsheared
WQ_SH = 335            # psum width in pixels per row-half, sheared (k=1)
W_UN = WQ_UN * C       # 672 elems
W_SH = WQ_SH * C       # 1005 elems
CHUNK0 = 512           # first psum chunk width (one full bank of fp32)

N_CORES = 8
SLOTS = 16


def split_sync_waits(nc, max_waits=1):
    n_split = 0
    for fn in nc.m.functions:
        for blk in fn.blocks:
            new_insts = []
            for inst in blk.instructions:
                si = inst.sync_info
                waits = list(si.on_wait) if (si and si.on_wait) else []
                if len(waits) > max_waits:
                    keep = waits[-max_waits:]
                    extra = waits[:-max_waits]
                    for j, w in enumerate(extra):
                        n_split += 1
                        nop = mybir.InstNoOp(
                            name=f"{inst.name}-waitsplit-{j}",
                            engine=inst.engine,
                            ins=[], outs=[],
                            sync_info=mybir.SyncInfo(on_wait=[w], on_update=[]),
                        )
                        new_insts.append(nop)
                    inst.sync_info = mybir.SyncInfo(on_wait=keep,
                                                    on_update=list(si.on_update or []))
                new_insts.append(inst)
            blk.instructions = new_insts
    return n_split


# ---------------------------------------------------------------- host math
def rotate_nearest_np(img, rad):
    K = img.shape[0]
    cos, sin = np.cos(rad), np.sin(rad)
    coords = np.arange(K, dtype=np.float32)
    yy, xx = np.meshgrid(coords, coords, indexing="ij")
    e = np.float32(K - 1)
    x_off = (e - (cos * e - sin * e)) * 0.5
    y_off = (e - (sin * e + cos * e)) * 0.5
    sx = cos * xx - sin * yy + x_off
    sy = sin * xx + cos * yy + y_off
    ix = np.round(sx).astype(np.int32)
    iy = np.round(sy).astype(np.int32)
    valid = (ix >= 0) & (ix < K) & (iy >= 0) & (iy < K)
    g = img[np.clip(iy, 0, K - 1), np.clip(ix, 0, K - 1)]
    return np.where(valid, g, np.float32(0.0))


def sample_taps(tbl_ch0, amt_b, ang_b):
    """-> (scale, ys, xs): tap rows/cols of the rotated kernel."""
    rad = np.float32(ang_b * math.pi / 180.0)
    ker = rotate_nearest_np(tbl_ch0[amt_b], rad)
    ys, xs = np.nonzero(ker)
    scale = float(ker[ys[0], xs[0]])
    return np.float32(scale), ys.astype(np.int64), xs.astype(np.int64)


def _span(v):
    return int(v.max() - v.min() + 1)


def span_options(ys, xs):
    """(span_unsheared, span_sheared): unsheared picks transpose; sheared
    picks k=+1 with optional flip (k=-1 equivalent)."""
    s_un = min(_span(xs), _span(ys))
    s_sh = min(_span(xs - ys), _span(xs + ys))
    return s_un, s_sh


def pack_slots(spans_un, spans_sh):
    """Partition 128 samples into 16 slots of 8 and pick slot modes
    (unsheared width 672 / sheared width 1005) minimizing
    sum(2 * width * max-span).  Local search with swaps + mode flips."""
    B = len(spans_un)
    n_slots = B // N_CORES

    def span_of(b, sh):
        return spans_sh[b] if sh else spans_un[b]

    def slot_cost(members, sh):
        w = W_SH if sh else W_UN
        return 2 * w * max(span_of(b, sh) for b in members)

    # init: solo-preference pools, sorted desc, chunked
    pref_sh = [b for b in range(B) if W_SH * spans_sh[b] < W_UN * spans_un[b]]
    pref_un = [b for b in range(B) if b not in pref_sh]
    pref_sh.sort(key=lambda b: -spans_sh[b])
    while len(pref_sh) % N_CORES:
        pref_un.append(pref_sh.pop())
    pref_un.sort(key=lambda b: -spans_un[b])
    slots = [pref_sh[i:i + N_CORES] for i in range(0, len(pref_sh), N_CORES)]
    modes = [True] * len(slots)
    slots += [pref_un[i:i + N_CORES] for i in range(0, len(pref_un), N_CORES)]
    modes += [False] * (n_slots - len(modes))

    costs = [slot_cost(m, s) for m, s in zip(slots, modes)]
    improved = True
    rounds = 0
    while improved and rounds < 60:
        improved = False
        rounds += 1
        for i in range(n_slots):
            for s in (True, False):
                if modes[i] != s:
                    c2 = slot_cost(slots[i], s)
                    if c2 < costs[i]:
                        modes[i], costs[i] = s, c2
                        improved = True
            for j in range(i + 1, n_slots):
                for a in range(N_CORES):
                    for b in range(N_CORES):
                        slots[i][a], slots[j][b] = slots[j][b], slots[i][a]
                        ci = slot_cost(slots[i], modes[i])
                        cj = slot_cost(slots[j], modes[j])
                        if ci + cj < costs[i] + costs[j]:
                            costs[i], costs[j] = ci, cj
                            improved = True
                        else:
                            slots[i][a], slots[j][b] = slots[j][b], slots[i][a]
    return slots, modes


def plan(kernels_table, amt, angles):
    """Full host plan: per-sample variants + slot schedule."""
    B = len(amt)
    tbl_ch0 = np.ascontiguousarray(kernels_table[:, :, :, 0])
    scales, taps = [], []
    for b in range(B):
        s, ys, xs = sample_taps(tbl_ch0, int(amt[b]), int(angles[b]))
        scales.append(s)
        taps.append((ys, xs))
    spans_un = [span_options(*taps[b])[0] for b in range(B)]
    spans_sh = [span_options(*taps[b])[1] for b in range(B)]

    slots, modes = pack_slots(spans_un, spans_sh)

    # order slots: interleave light/heavy by PE cost (light first)
    def pe_cost(i):
        w = W_SH if modes[i] else W_UN
        sp = [spans_sh[b] if modes[i] else spans_un[b] for b in slots[i]]
        return 2 * w * max(sp)
    order = sorted(range(len(slots)), key=pe_cost)
    ileave = []
    lo, hi = 0, len(order) - 1
    while lo <= hi:
        ileave.append(order[lo])
        if lo != hi:
            ileave.append(order[hi])
        lo += 1
        hi -= 1
    slots = [slots[i] for i in ileave]
    modes = [modes[i] for i in ileave]

    # per-sample final variant given slot mode
    samples = {}
    for j, (members, sh) in enumerate(zip(slots, modes)):
        for c, b in enumerate(members):
            ys, xs = taps[b]
            if sh:
                tr = 0
                fl = _span(xs + ys) < _span(xs - ys)
                k = 1
            else:
                tr = _span(ys) < _span(xs)
                fl = False
                k = 0
            ky, kx = (xs, ys) if tr else (ys, xs)
            if fl:
                kx = 30 - kx
            kxp = kx - k * ky
            m = int(kxp.min())
            G = int(kxp.max()) - m + 1
            D = (223 - m) if sh else (15 - m)
            samples[b] = dict(slot=j, core=c, tr=bool(tr), fl=bool(fl), k=k,
                              m=m, G=G, D=D, ky=ky, kxp=kxp,
                              scale=scales[b])

    gmax = []
    for j, (members, sh) in enumerate(zip(slots, modes)):
        gmax.append(max(samples[b]["G"] for b in members))

    meta = dict(slots=slots, modes=modes, gmax=gmax, samples=samples)

    # static widths
    sh_idx = [j for j in range(len(slots)) if modes[j]]
    un_idx = [j for j in range(len(slots)) if not modes[j]]
    meta["sh_idx"] = sh_idx
    meta["un_idx"] = un_idx
    gmax_sh = max([gmax[j] for j in sh_idx], default=1)
    gmax_un = max([gmax[j] for j in un_idx], default=1)
    meta["XW_SH"] = 336 + 3 * (gmax_sh - 1) + W_SH
    meta["XW_UN"] = 3 * (gmax_un - 1) + W_UN
    meta["col_base"] = np.concatenate([[0], np.cumsum([g * 224 for g in gmax])])[:-1]
    meta["totcols"] = int(sum(g * 224 for g in gmax))
    return meta


# ------------------------------------------------------------- host tensors
def prepare_host(x, kernels_table, amt, angles, n_cores=N_CORES):
    import ml_dtypes

    B = x.shape[0]
    meta = plan(kernels_table, amt, angles)
    slots, modes, gmax = meta["slots"], meta["modes"], meta["gmax"]
    samples = meta["samples"]
    XW_SH, XW_UN = meta["XW_SH"], meta["XW_UN"]
    n_sh, n_un = len(meta["sh_idx"]), len(meta["un_idx"])
    # slot j -> index within its dram tensor
    slot_sub = {}
    for i, j in enumerate(meta["sh_idx"]):
        slot_sub[j] = i
    for i, j in enumerate(meta["un_idx"]):
        slot_sub[j] = i
    meta["slot_sub"] = slot_sub

    in_maps = []
    for c in range(n_cores):
        ximg_sh = np.zeros((max(n_sh, 1), H, XW_SH), ml_dtypes.bfloat16)
        ximg_un = np.zeros((max(n_un, 1), H, XW_UN), ml_dtypes.bfloat16)
        wt = np.zeros((128, meta["totcols"]), np.uint8)  # fp8e4 bit pattern
        for j in range(len(slots)):
            b = slots[j][c]
            sp = samples[b]
            G, D, k = sp["G"], sp["D"], sp["k"]
            # variant image: scaled, transposed, flipped
            img = x[b].astype(np.float32) * sp["scale"]
            if sp["tr"]:
                img = img.transpose(1, 0, 2)
            if sp["fl"]:
                img = img[:, ::-1, :]
            img = np.ascontiguousarray(img).reshape(H, W * C)
            xw = XW_SH if modes[j] else XW_UN
            dst = ximg_sh[slot_sub[j]] if modes[j] else ximg_un[slot_sub[j]]
            # dst[y, 3u'+ch] = img[y, 3*(u' + k*y - D)+ch] clipped to [0,224)
            if k == 0:
                lo = 3 * D
                s0, s1 = max(0, lo), min(xw, lo + W * C)
                if s1 > s0:
                    dst[:, s0:s1] = img[:, s0 - lo:s1 - lo].astype(ml_dtypes.bfloat16)
            else:
                for y in range(H):
                    lo = 3 * (D - k * y)
                    s0, s1 = max(0, lo), min(xw, lo + W * C)
                    if s1 > s0:
                        dst[y, s0:s1] = img[y, s0 - lo:s1 - lo].astype(ml_dtypes.bfloat16)
            # weights: fp8e4 1.0 has bit pattern 0x38 (exp bias 7)
            ky, kxp, m = sp["ky"], sp["kxp"], sp["m"]
            cb = int(meta["col_base"][j])
            p = np.arange(128)[:, None]
            o = np.arange(112)[None, :]
            for g in range(G):
                rows = ky[kxp == m + g]
                if len(rows) == 0:
                    continue
                w0 = np.isin(p - o + 15, rows)
                w1 = np.isin(p - o - 1, rows)
                wt[:, cb + 224 * g:cb + 224 * g + 112][w0] = 0x38
                wt[:, cb + 224 * g + 112:cb + 224 * (g + 1)][w1] = 0x38
        in_maps.append({
            "ximg_sh": ximg_sh,
            "ximg_un": ximg_un,
            "wt": wt.view(ml_dtypes.float8_e4m3),
        })
    return meta, in_maps


# ---------------------------------------------------------------- device IR
WT_DTYPE_NAME = "float8e4"


def build_program(meta):
    import concourse.bacc as bacc
    from concourse.tile import TileContext

    bf16 = mybir.dt.bfloat16
    wdt = getattr(mybir.dt, WT_DTYPE_NAME)
    slots, modes, gmax = meta["slots"], meta["modes"], meta["gmax"]
    slot_sub = meta["slot_sub"]
    n_sh, n_un = len(meta["sh_idx"]), len(meta["un_idx"])

    nc = bacc.Bacc("TRN2")
    ximg_sh = nc.dram_tensor("ximg_sh", [max(n_sh, 1), H, meta["XW_SH"]], bf16,
                             kind="ExternalInput")
    ximg_un = nc.dram_tensor("ximg_un", [max(n_un, 1), H, meta["XW_UN"]], bf16,
                             kind="ExternalInput")
    wt_d = nc.dram_tensor("wt", [128, meta["totcols"]], wdt, kind="ExternalInput")
    out_sh = nc.dram_tensor("out_sh", [max(n_sh, 1), H, W_SH], bf16,
                            kind="ExternalOutput")
    out_un = nc.dram_tensor("out_un", [max(n_un, 1), H, W_UN], bf16,
                            kind="ExternalOutput")

    with TileContext(nc) as tc:
        with tc.tile_pool(name="img", bufs=4) as ipool, \
             tc.tile_pool(name="wtp", bufs=3) as wpool, \
             tc.tile_pool(name="res", bufs=4) as rpool, \
             tc.tile_pool(name="ps0", bufs=2, space="PSUM") as pp00, \
             tc.tile_pool(name="ps1", bufs=2, space="PSUM") as pp01, \
             tc.tile_pool(name="ps2", bufs=2, space="PSUM") as pp10, \
             tc.tile_pool(name="ps3", bufs=2, space="PSUM") as pp11:
            psum_pools = [[pp00, pp01], [pp10, pp11]]
            for j in range(len(slots)):
                sh = modes[j]
                G = gmax[j]
                Wp = W_SH if sh else W_UN
                TW = 3 * (G - 1) + Wp
                off0 = 336 if sh else 0
                xsrc = ximg_sh if sh else ximg_un
                js = slot_sub[j]
                cb = int(meta["col_base"][j])
                chunks = [(0, CHUNK0), (CHUNK0, Wp - CHUNK0)]

                t0 = ipool.tile([128, TW], bf16, tag="t0", name="t0")
                t1 = ipool.tile([128, TW], bf16, tag="t1", name="t1")
                nc.sync.dma_start(out=t0, in_=xsrc[js, 0:128, off0:off0 + TW])
                nc.sync.dma_start(out=t1, in_=xsrc[js, 96:224, 0:TW])
                wtt = wpool.tile([128, 224 * G], wdt, tag="wt", name="wtt")
                nc.sync.dma_start(out=wtt, in_=wt_d[:, cb:cb + 224 * G])

                tiles = [t0, t1]
                rts = []
                for hb in (0, 1):
                    rt = rpool.tile([112, Wp], bf16, tag=f"r{hb}", name=f"r{hb}")
                    rts.append(rt)
                    for ci, (c0, cw) in enumerate(chunks):
                        ps = psum_pools[hb][ci].tile(
                            [112, CHUNK0], mybir.dt.float32,
                            tag=f"ps{hb}{ci}", name=f"ps{hb}{ci}")
                        for g in range(G):
                            lhsT = wtt[:, 224 * g + 112 * hb:224 * g + 112 * hb + 112]
                            nc.tensor.matmul(ps[:, 0:cw], lhsT=lhsT,
                                             rhs=tiles[hb][:, 3 * g + c0:3 * g + c0 + cw],
                                             start=(g == 0), stop=(g == G - 1))
                        if ci == 0:
                            nc.scalar.activation(
                                out=rt[:, c0:c0 + cw], in_=ps[:, 0:cw],
                                func=mybir.ActivationFunctionType.Copy)
                        else:
                            nc.vector.tensor_copy(out=rt[:, c0:c0 + cw],
                                                  in_=ps[:, 0:cw])
                odst = out_sh if sh else out_un
                for hb in (0, 1):
                    nc.scalar.dma_start(out=odst[js, 112 * hb:112 * (hb + 1), :],
                                        in_=rts[hb])
    return nc


def run_cores(meta, in_maps, trace=False):
    from concourse.bass_utils import run_bass_kernel_spmd

    nc = build_program(meta)
    nc.compile()
    split_sync_waits(nc)
    res = run_bass_kernel_spmd(nc, in_maps, core_ids=list(range(len(in_maps))),
                               trace=trace)
    return res


# ------------------------------------------------------------------ unshard
def unshard(meta, results):
    slots, modes = meta["slots"], meta["modes"]
    samples = meta["samples"]
    slot_sub = meta["slot_sub"]
    B = sum(len(m) for m in slots)
    out = np.zeros((B, H, W, C), np.float32)
    r_idx = np.arange(H)
    # sheared per-row pixel window start: r<112 -> 111-r ; r>=112 -> 223-r
    q0 = np.where(r_idx < 112, 111 - r_idx, 223 - r_idx)
    for c, r in enumerate(results):
        o_sh = np.asarray(r["out_sh"]).astype(np.float32)
        o_un = np.asarray(r["out_un"]).astype(np.float32)
        for j in range(len(slots)):
            b = slots[j][c]
            sp = samples[b]
            js = slot_sub[j]
            if modes[j]:
                arr = o_sh[js]  # [224, 1005]
                cols = (3 * q0)[:, None] + np.arange(W * C)[None, :]
                img = np.take_along_axis(arr, cols, axis=1)
            else:
                img = o_un[js]
            img = img.reshape(H, W, C)
            if sp["fl"]:
                img = img[:, ::-1, :]
            if sp["tr"]:
                img = img.transpose(1, 0, 2)
            out[b] = img
    return out


def kernel(x, kernels_table, amt, angles):
    x = np.asarray(x, np.float32)
    kernels_table = np.asarray(kernels_table, np.float32)
    amt = np.asarray(amt)
    angles = np.asarray(angles)
    meta, in_maps = prepare_host(x, kernels_table, amt, angles)
    res = run_cores(meta, in_maps)
    return unshard(meta, res.results)
